# revision 44
# baseline (speedup 1.0000x reference)
"""Bass/Tile TRN2 kernel for BiasMultiheadAttention (B=4, S=2048, D=512, H=8).

Single fused NEFF across 8 cores, one attention head per core. Wall-clock is
dominated by host<->device transfer over the axon tunnel (~70 MB/s, ~10 ms
fixed cost per transfer), so the kernel minimizes both bytes AND transfer
count:

  - ALL inputs travel in ONE packed uint8 blob per core (8 device_put calls
    total): int8 bias (per-row scales, magic-number quantization on host,
    ~165 ms for 128 MB) + bf16 x shard + f32 weights pre-tiled to SBUF
    layout. Blob c is quantized/packed while blob c-1 streams.
  - x is shipped sharded (1/8 per core) and AllGathered on device.
  - the out-projection runs in the same NEFF after an AllToAll pivot of the
    per-head outputs; a final AllGather replicates the bf16 result so the
    host fetches exactly ONE 8 MB shard.
  - the PE-transpose identity matrix is a Const tensor embedded in the NEFF.
  - the jitted shard_map executable is cached; donated output buffers are
    recycled between calls.

Math layout per core (head h), matmuls in f32r:
  QT = (SCALE*Wq_h) @ x^T + SCALE*bq   -> [64, B*S]   (dh on partitions)
  KT = Wk_h @ x^T + bk                 -> [64, B*S]
  V  = x @ Wv_h^T + bv                 -> per k-tile [128, 65] with ones col
  S^T[k,q] = KT_tile^T @ QT_chunk      (PSUM, per batch)
  S^T += bias_h^T (DVE add; bias^T tiles built on-device from int8*scale)
  P^T = exp(S^T)                       (ACT, no max-subtraction: scores O(1))
  O^T|sums = (V|1)^T @ P^T             (PSUM accum over k tiles)
  O^T norm = O^T * (1/sums) broadcast
  AllToAll over q-blocks -> core c holds O^T[:, c*1024:(c+1)*1024] all heads
  out rows = O_rows @ w_out^T + b_out  (b_out via K=1 matmul), AllGather.
"""

import sys

for _p in ("/opt/trn_rl_repo",):
    if _p not in sys.path:
        sys.path.append(_p)

import numpy as np
import ml_dtypes

import concourse.bass as bass
import concourse.mybir as mybir
import concourse.tile as tile
from concourse import bacc

F32 = mybir.dt.float32
F32R = mybir.dt.float32r
BF16 = mybir.dt.bfloat16
I8 = mybir.dt.int8
U8 = mybir.dt.uint8
EXPF = mybir.ActivationFunctionType.Exp
IDENTF = mybir.ActivationFunctionType.Identity

N_CORES = 8
B, S, D = 4, 2048, 512
H, DH = 8, 64
SCALE = DH ** -0.5
ROWS = B * S            # 8192
RPC = ROWS // N_CORES   # 1024 rows per core (= one q-block)
RC = 512                # row chunk for projections
N_RC = ROWS // RC       # 16
FT = D // 128           # 4 feature tiles
KT_PER_B = S // 128     # 16 k-tiles per batch
QH = S // 2             # 1024, q processed in halves
QC = 512                # q chunk (one PSUM bank wide)
N_QC_H = QH // QC       # 2
RG = [list(range(N_CORES))]

# ---- packed input blob layout (bytes, all sections 4-byte aligned) ----
OFF_BIAS = 0                          # int8 [S, S] row-major [q, k]
SZ_BIAS = S * S
OFF_BSC = OFF_BIAS + SZ_BIAS          # f32 [128, 16] tiled: [p][t], q=t*128+p
SZ_BSC = S * 4
OFF_XS = OFF_BSC + SZ_BSC             # bf16 [RPC, D] row-major
SZ_XS = RPC * D * 2
OFF_WQK = OFF_XS + SZ_XS              # bf16 [128, FT, 128]: [p][ft][m]
SZ_WQK = D * 2 * DH * 2
OFF_WV = OFF_WQK + SZ_WQK             # bf16 [128, FT, 64]: [p][ft][m]
SZ_WV = D * DH * 2
OFF_BQK = OFF_WV + SZ_WV              # f32 [128]
SZ_BQK = 2 * DH * 4
OFF_BV = OFF_BQK + SZ_BQK             # f32 [64]
SZ_BV = DH * 4
OFF_WOS = OFF_BV + SZ_BV              # bf16 [64, 512] rows h*64.. of woT
SZ_WOS = DH * D * 2
OFF_BO = OFF_WOS + SZ_WOS             # f32 [512]
SZ_BO = D * 4
BYTES_PC = OFF_BO + SZ_BO


def build_fused():
    nc = bacc.Bacc("TRN2", target_bir_lowering=False, debug=False,
                   enable_asserts=False, num_devices=N_CORES)

    blob = nc.dram_tensor("blob", [1, BYTES_PC], U8, kind="ExternalInput")
    out = nc.dram_tensor("out", [ROWS, D], BF16, kind="ExternalOutput")
    ident = nc.inline_tensor(np.eye(128, dtype=np.float32), name="identc")

    def sec(off, size, dt):
        return blob.ap()[:, off:off + size].bitcast(dt)

    with tile.TileContext(nc) as tc:
        from contextlib import ExitStack
        with ExitStack() as stk:
            dram = stk.enter_context(
                tc.tile_pool(name="dram", bufs=1, space="DRAM"))
            xt_loc = dram.tile([D, RPC], F32R, tag="xt_loc")
            xt_all = dram.tile([N_CORES * D, RPC], F32R, tag="xt_all",
                               addr_space="Shared")
            wo_loc = dram.tile([DH, D], BF16, tag="wo_loc")
            wo_all = dram.tile([D, D], BF16, tag="wo_all",
                               addr_space="Shared")
            ot_loc = dram.tile([N_CORES * DH, RPC], F32R, tag="ot_loc")
            ot_a2a = dram.tile([N_CORES * DH, RPC], F32R, tag="ot_a2a")
            out_loc = dram.tile([RPC, D], BF16, tag="out_loc")
            out_all = dram.tile([ROWS, D], BF16, tag="out_all",
                                addr_space="Shared")

            persist = stk.enter_context(tc.tile_pool(name="persist", bufs=1))
            QKT = persist.tile([2 * DH, ROWS], F32R, tag="QKT")
            KTx = persist.tile([DH, ROWS], F32R, tag="KTx")
            Vaug = persist.tile([128, B * KT_PER_B, DH + 1], F32R, tag="Vaug")
            wqk_sb = persist.tile([128, FT * 2 * DH], F32R, tag="wqk")
            wv_sb = persist.tile([128, FT * DH], F32R, tag="wv")
            bqk_sb = persist.tile([2 * DH, 1], F32, tag="bqk")
            bv_sb = persist.tile([DH, 1], F32, tag="bv")
            idf_sb = persist.tile([128, 128], F32R, tag="idf")
            bsc_sb = persist.tile([128, S // 128], F32, tag="bsc")
            ones64 = persist.tile([DH + 1, 128], F32R, tag="ones64")
            ones1 = persist.tile([1, 128], F32R, tag="ones1")
            wo_sb = persist.tile([128, FT, D], F32R, tag="wo_sb")
            bo_sb = persist.tile([1, D], F32R, tag="bo_sb")
            # bias^T tiles for BOTH halves: [half*16+kt] -> [128 k, 1024 q]
            bias_t = [persist.tile([128, QH], BF16, tag=f"bias_t{i}",
                                   name=f"bias_t{i}")
                      for i in range(2 * KT_PER_B)]

            nc.gpsimd.memset(ones64[DH:DH + 1, :].bitcast(F32), 1.0)
            nc.gpsimd.memset(ones1[:].bitcast(F32), 1.0)
            nc.gpsimd.memset(Vaug[:, :, DH:DH + 1].bitcast(F32), 1.0)
            with tc.tile_pool(name="wstg", bufs=1) as wstg:
                wqk_bf = wstg.tile([128, FT * 2 * DH], BF16, tag="wqk_bf")
                wv_bf = wstg.tile([128, FT * DH], BF16, tag="wv_bf")
                nc.sync.dma_start(
                    wqk_bf[:],
                    sec(OFF_WQK, SZ_WQK, BF16).rearrange(
                        "o (p f) -> p (f o)", p=128))
                nc.sync.dma_start(
                    wv_bf[:],
                    sec(OFF_WV, SZ_WV, BF16).rearrange(
                        "o (p f) -> p (f o)", p=128))
                nc.scalar.copy(wqk_sb[:], wqk_bf[:])
                nc.scalar.copy(wv_sb[:], wv_bf[:])
            nc.sync.dma_start(
                bqk_sb[:],
                sec(OFF_BQK, SZ_BQK, F32).rearrange("o (p f) -> p (f o)",
                                                    p=2 * DH))
            nc.sync.dma_start(
                bv_sb[:],
                sec(OFF_BV, SZ_BV, F32).rearrange("o (p f) -> p (f o)", p=DH))
            nc.sync.dma_start(idf_sb[:], ident.ap().bitcast(F32R))
            nc.sync.dma_start(
                bsc_sb[:],
                sec(OFF_BSC, SZ_BSC, F32).rearrange("o (p f) -> p (f o)",
                                                    p=128))
            nc.sync.dma_start(bo_sb[:], sec(OFF_BO, SZ_BO, F32R))

            # ---- stage A: transpose local x shard, AllGather; wo AllGather
            with tc.tile_pool(name="xsp", bufs=2) as xsp, \
                 tc.tile_pool(name="xtsb", bufs=1) as xtsb, \
                 tc.tile_pool(name="xt_ps", bufs=4, space="PSUM") as xt_ps:
                xt_sb = xtsb.tile([128, FT, RPC], F32R, tag="xt_sb")
                for rt in range(RPC // 128):
                    xs_sb = xsp.tile([128, D], BF16, tag="xs_sb")
                    nc.sync.dma_start(
                        xs_sb[:],
                        sec(OFF_XS + rt * 128 * D * 2, 128 * D * 2, BF16)
                        .rearrange("o (p f) -> p (f o)", p=128))
                    xs_f = xsp.tile([128, D], F32R, tag="xs_f")
                    nc.scalar.copy(xs_f[:], xs_sb[:])
                    for ft in range(FT):
                        tr = xt_ps.tile([128, 128], F32R, tag="xtr")
                        nc.tensor.transpose(
                            tr[:], xs_f[:, ft * 128:(ft + 1) * 128],
                            idf_sb[:])
                        nc.vector.tensor_copy(
                            xt_sb[:, ft, rt * 128:(rt + 1) * 128], tr[:])
                nc.sync.dma_start(
                    xt_loc[:].rearrange("(t p) r -> p t r", p=128), xt_sb[:])
                nc.sync.dma_start(
                    wo_loc[:],
                    sec(OFF_WOS, SZ_WOS, BF16).rearrange(
                        "o (p f) -> p (f o)", p=DH))

            nc.gpsimd.collective_compute(
                "AllGather", mybir.AluOpType.bypass, replica_groups=RG,
                ins=[xt_loc.opt()], outs=[xt_all.opt()])
            nc.gpsimd.collective_compute(
                "AllGather", mybir.AluOpType.bypass, replica_groups=RG,
                ins=[wo_loc.opt()], outs=[wo_all.opt()])

            with tc.tile_pool(name="wostg", bufs=1) as wostg:
                wo_bf = wostg.tile([128, FT, D], BF16, tag="wo_bf")
                nc.sync.dma_start(
                    wo_bf[:],
                    wo_all[:].rearrange("(t p) m -> p t m", p=128))
                nc.scalar.copy(wo_sb[:], wo_bf[:])

            # ---- stage B: bias^T tiles on device (both halves up front)
            with tc.tile_pool(name="stgp", bufs=3) as stgp, \
                 tc.tile_pool(name="stgf", bufs=2) as stgf, \
                 tc.tile_pool(name="btr_ps", bufs=4, space="PSUM") as btr_ps:
                for half in range(2):
                    q0 = half * QH
                    for qt in range(QH // 128):
                        qt_g = half * (QH // 128) + qt
                        stg = stgp.tile([128, S], I8, tag="stg")
                        nc.sync.dma_start(
                            stg[:],
                            sec(OFF_BIAS + (q0 + qt * 128) * S, 128 * S, I8)
                            .rearrange("o (p f) -> p (f o)", p=128))
                        stf = stgf.tile([128, S], F32R, tag="stf")
                        nc.scalar.activation(stf[:], stg[:], IDENTF,
                                             scale=bsc_sb[:, qt_g:qt_g + 1])
                        for kt in range(KT_PER_B):
                            btr = btr_ps.tile([128, 128], F32R, tag="btr")
                            nc.tensor.transpose(
                                btr[:], stf[:, kt * 128:(kt + 1) * 128],
                                idf_sb[:])
                            nc.vector.tensor_copy(
                                bias_t[half * KT_PER_B + kt]
                                [:, qt * 128:(qt + 1) * 128], btr[:])

            # ---- stage C: QKV projections
            with tc.tile_pool(name="xtp", bufs=2) as xtp, \
                 tc.tile_pool(name="vtsb", bufs=2) as vtsb, \
                 tc.tile_pool(name="qk_ps", bufs=3, space="PSUM") as qk_ps, \
                 tc.tile_pool(name="v_ps", bufs=2, space="PSUM") as v_ps, \
                 tc.tile_pool(name="tr_ps", bufs=3, space="PSUM") as tr_ps:
                for rc in range(N_RC):
                    cblk, off = divmod(rc, RPC // RC)
                    off *= RC
                    xt = xtp.tile([128, FT, RC], F32R, tag="xt")
                    nc.sync.dma_start(
                        xt[:],
                        xt_all[cblk * D:(cblk + 1) * D, off:off + RC]
                        .rearrange("(t p) r -> p t r", p=128))

                    ps = qk_ps.tile([2 * DH, RC], F32, tag="qk")
                    for ft in range(FT):
                        nc.tensor.matmul(
                            ps[:], wqk_sb[:, ft * 2 * DH:(ft + 1) * 2 * DH],
                            xt[:, ft, :],
                            start=(ft == 0), stop=(ft == FT - 1))
                    nc.scalar.activation(
                        QKT[:, rc * RC:(rc + 1) * RC], ps[:], IDENTF,
                        bias=bqk_sb[:])
                    nc.sync.dma_start(
                        KTx[:, rc * RC:(rc + 1) * RC],
                        QKT[DH:2 * DH, rc * RC:(rc + 1) * RC])

                    vt_ps = v_ps.tile([DH, RC], F32, tag="vt")
                    for ft in range(FT):
                        nc.tensor.matmul(
                            vt_ps[:], wv_sb[:, ft * DH:(ft + 1) * DH],
                            xt[:, ft, :],
                            start=(ft == 0), stop=(ft == FT - 1))
                    vt_sb = vtsb.tile([DH, RC], F32R, tag="vt_sb")
                    nc.scalar.activation(vt_sb[:], vt_ps[:], IDENTF,
                                         bias=bv_sb[:])
                    for sub in range(RC // 128):
                        tr = tr_ps.tile([128, DH], F32R, tag="tr")
                        nc.tensor.transpose(
                            tr[:], vt_sb[:, sub * 128:(sub + 1) * 128],
                            idf_sb[0:DH, 0:DH])
                        rt = rc * (RC // 128) + sub
                        b_i, kt_i = divmod(rt, KT_PER_B)
                        nc.vector.tensor_copy(
                            Vaug[:, b_i * KT_PER_B + kt_i, 0:DH], tr[:])

            # ---- stage D: attention
            with tc.tile_pool(name="esb", bufs=2) as esb, \
                 tc.tile_pool(name="ssb", bufs=2) as ssb, \
                 tc.tile_pool(name="osb", bufs=2) as osb, \
                 tc.tile_pool(name="onsb", bufs=1) as onsb, \
                 tc.tile_pool(name="sc_ps", bufs=3, space="PSUM") as sc_ps, \
                 tc.tile_pool(name="ot_ps", bufs=2, space="PSUM") as ot_ps:
                for half in range(2):
                    q0 = half * QH
                    for b_i in range(B):
                        qoff = b_i * S + q0
                        otps = [ot_ps.tile([DH + 1, QC], F32, tag="ot",
                                           name=f"ot_{half}_{b_i}_{qc}")
                                for qc in range(N_QC_H)]

                        def emit_av(ktp, e_sb):
                            for j in range(2):
                                kt = 2 * ktp + j
                                for qc in range(N_QC_H):
                                    nc.tensor.matmul(
                                        otps[qc][:],
                                        Vaug[:, b_i * KT_PER_B + kt, :],
                                        e_sb[:, j * QH + qc * QC:
                                             j * QH + (qc + 1) * QC],
                                        start=(ktp == 0 and j == 0),
                                        stop=(ktp == KT_PER_B // 2 - 1
                                              and j == 1),
                                        skip_group_check=True)

                        pending = None
                        for ktp in range(KT_PER_B // 2):
                            e_sb = esb.tile([128, 2 * QH], F32R, tag="e")
                            s_sb = ssb.tile([128, 2 * QH], F32, tag="s",
                                            name="s_sb")
                            for j in range(2):
                                kt = 2 * ktp + j
                                koff = b_i * S + kt * 128
                                ps = sc_ps.tile([128, QH], F32, tag="sc")
                                for qc in range(N_QC_H):
                                    nc.tensor.matmul(
                                        ps[:, qc * QC:(qc + 1) * QC],
                                        KTx[:, koff:koff + 128],
                                        QKT[0:DH, qoff + qc * QC:
                                            qoff + (qc + 1) * QC],
                                        start=True, stop=True,
                                        skip_group_check=True)
                                nc.vector.tensor_add(
                                    s_sb[:, j * QH:(j + 1) * QH], ps[:],
                                    bias_t[half * KT_PER_B + kt][:])
                            nc.scalar.activation(e_sb[:], s_sb[:], EXPF)
                            if pending is not None:
                                emit_av(*pending)
                            pending = (ktp, e_sb)
                        if pending is not None:
                            emit_av(*pending)

                        # normalize: O^T[:64] * (1/sums); sums live in row 64
                        o_sb = osb.tile([DH + 1, QH], F32R, tag="o")
                        for qc in range(N_QC_H):
                            nc.vector.tensor_copy(
                                o_sb[:, qc * QC:(qc + 1) * QC], otps[qc][:])
                        with nc.allow_low_precision(
                                reason="softmax denom recip in f32r is fine"):
                            nc.vector.reciprocal(o_sb[DH:DH + 1, :],
                                                 o_sb[DH:DH + 1, :])
                        bc = sc_ps.tile([DH, QH], F32, tag="sc", name="bc")
                        for qc in range(N_QC_H):
                            nc.tensor.matmul(
                                bc[:, qc * QC:(qc + 1) * QC],
                                ones64[DH:DH + 1, 0:DH],
                                o_sb[DH:DH + 1, qc * QC:(qc + 1) * QC],
                                start=True, stop=True)
                        on_sb = onsb.tile([DH, QH], F32, tag="on")
                        nc.vector.tensor_mul(on_sb[:], o_sb[0:DH, :], bc[:])
                        cblk = 2 * b_i + half
                        nc.sync.dma_start(
                            ot_loc[cblk * DH:(cblk + 1) * DH, :],
                            on_sb[:].bitcast(F32R))

            # ---- stage E: AllToAll q-blocks <-> heads, then out projection
            nc.gpsimd.collective_compute(
                "AllToAll", mybir.AluOpType.bypass, replica_groups=RG,
                ins=[ot_loc.opt()], outs=[ot_a2a.opt()])

            with tc.tile_pool(name="otsb2", bufs=1) as otsb2, \
                 tc.tile_pool(name="res", bufs=3) as res, \
                 tc.tile_pool(name="p2_ps", bufs=4, space="PSUM") as p2_ps:
                ot_sb2 = otsb2.tile([128, FT, RPC], F32R, tag="ot_sb2")
                nc.sync.dma_start(
                    ot_sb2[:],
                    ot_a2a[:].rearrange("(t p) r -> p t r", p=128))
                for rt in range(RPC // 128):
                    ps = p2_ps.tile([128, D], F32, tag="ps")
                    nc.tensor.matmul(ps[:], ones1[:], bo_sb[:],
                                     start=True, stop=False)
                    for ft in range(FT):
                        nc.tensor.matmul(
                            ps[:], ot_sb2[:, ft, rt * 128:(rt + 1) * 128],
                            wo_sb[:, ft, :],
                            start=False, stop=(ft == FT - 1))
                    r_sb = res.tile([128, D], BF16, tag="r")
                    nc.scalar.copy(r_sb[:], ps[:])
                    nc.sync.dma_start(
                        out_loc[rt * 128:(rt + 1) * 128, :], r_sb[:])

            # replicate the result so the host fetches a single shard
            nc.gpsimd.collective_compute(
                "AllGather", mybir.AluOpType.bypass, replica_groups=RG,
                ins=[out_loc.opt()], outs=[out_all.opt()])
            nc.sync.dma_start(out.ap(), out_all[:])

    nc.compile()
    return nc


_CACHE = {}


def _make_runner(nc, n_cores=N_CORES):
    import jax
    from jax.experimental.shard_map import shard_map
    from jax.sharding import Mesh, PartitionSpec, NamedSharding
    from concourse.bass2jax import (_bass_exec_p, partition_id_tensor,
                                    install_neuronx_cc_hook)

    install_neuronx_cc_hook()
    partition_name = (nc.partition_id_tensor.name
                      if nc.partition_id_tensor else None)

    in_names, out_names, out_avals, zero_outs = [], [], [], []
    for alloc in nc.m.functions[0].allocations:
        if not isinstance(alloc, mybir.MemoryLocationSet):
            continue
        name = alloc.memorylocations[0].name
        if alloc.kind == "ExternalInput":
            if name != partition_name:
                in_names.append(name)
        elif alloc.kind == "ExternalOutput":
            shape = tuple(alloc.tensor_shape)
            dtype = mybir.dt.np(alloc.dtype)
            out_names.append(name)
            out_avals.append(jax.core.ShapedArray(shape, dtype))
            zero_outs.append(np.zeros((n_cores * shape[0], *shape[1:]), dtype))
    n_params = len(in_names)
    n_outs = len(out_names)
    bind_in_names = list(in_names) + list(out_names)
    if partition_name is not None:
        bind_in_names.append(partition_name)
    donate = tuple(range(n_params, n_params + n_outs))

    def _body(*args):
        operands = list(args)
        if partition_name is not None:
            operands.append(partition_id_tensor())
        outs = _bass_exec_p.bind(
            *operands,
            out_avals=tuple(out_avals),
            in_names=tuple(bind_in_names),
            out_names=tuple(out_names),
            lowering_input_output_aliases=(),
            sim_require_finite=True,
            sim_require_nnan=True,
            nc=nc,
        )
        return tuple(outs)

    devices = jax.devices()[:n_cores]
    assert len(devices) == n_cores
    mesh = Mesh(np.asarray(devices), ("core",))
    in_specs = (PartitionSpec("core"),) * (n_params + n_outs)
    out_specs = (PartitionSpec("core"),) * n_outs
    sharded = jax.jit(
        shard_map(_body, mesh=mesh, in_specs=in_specs, out_specs=out_specs,
                  check_rep=False),
        donate_argnums=donate, keep_unused=True)

    sharding = NamedSharding(mesh, PartitionSpec("core"))
    state = {"donate": zero_outs}

    def run(global_inputs):
        args = [global_inputs[nm] for nm in in_names]
        outs = sharded(*args, *state["donate"])
        # recycle the device-resident outputs as next call's donated buffers
        # (they are fully overwritten by the kernel; saves an upload)
        state["donate"] = list(outs)
        return dict(zip(out_names, outs))

    run.sharding = sharding
    run.devices = devices
    return run


def _get_runner():
    if "runner" not in _CACHE:
        nc = build_fused()
        _CACHE["runner"] = _make_runner(nc)
    return _CACHE["runner"]


def _scratch(name, shape, dtype):
    key = ("scratch", name)
    if key not in _CACHE:
        _CACHE[key] = np.empty(shape, dtype)
    return _CACHE[key]


def kernel(x, attn_bias, w_in, b_in, w_out, b_out):
    import jax

    x = np.asarray(x, dtype=np.float32)
    attn_bias = np.asarray(attn_bias, dtype=np.float32)
    w_in = np.asarray(w_in, dtype=np.float32)
    b_in = np.asarray(b_in, dtype=np.float32)
    w_out = np.asarray(w_out, dtype=np.float32)
    b_out = np.asarray(b_out, dtype=np.float32)

    run = _get_runner()

    # --- shared (cheap) weight prep, pre-tiled to the SBUF layouts
    wq = w_in[0:D].reshape(H, DH, D) * SCALE
    wk = w_in[D:2 * D].reshape(H, DH, D)
    # per head: [512 in, 128 out] -> tiled [p=128][ft=4][m=128]
    wqkT = np.concatenate([wq, wk], axis=1).transpose(0, 2, 1)  # [H, D, 128]
    wqkT_t = np.ascontiguousarray(wqkT.reshape(H, FT, 128, 2 * DH)
                                  .transpose(0, 2, 1, 3)
                                  .astype(ml_dtypes.bfloat16))
    wvT = w_in[2 * D:3 * D].reshape(H, DH, D).transpose(0, 2, 1)  # [H, D, 64]
    wvT_t = np.ascontiguousarray(wvT.reshape(H, FT, 128, DH)
                                 .transpose(0, 2, 1, 3)
                                 .astype(ml_dtypes.bfloat16))
    bqk = np.concatenate(
        [b_in[0:D].reshape(H, DH) * SCALE, b_in[D:2 * D].reshape(H, DH)],
        axis=1)                                                 # [H, 128]
    bv = b_in[2 * D:3 * D].reshape(H, DH)
    woT = np.ascontiguousarray(w_out.T).reshape(H, DH, D).astype(
        ml_dtypes.bfloat16)                                     # [H, 64, 512]
    bo = b_out.reshape(D)

    b2 = attn_bias.reshape(H * S, S)
    x2 = x.reshape(ROWS, D)
    tmp = _scratch("tmp", (S, S), np.float32)
    blob = _scratch("blob", (N_CORES, BYTES_PC), np.uint8)
    MAGIC = np.float32(3 * 2.0 ** 22)

    # --- per-core: quantize bias+x chunk c + pack blob c, then start its
    # upload; chunk c streams through the tunnel while c+1 is prepared.
    chunks = []
    for c in range(N_CORES):
        bc = blob[c]
        blk = b2[c * S:(c + 1) * S]
        mxc = np.maximum(blk.max(axis=1), -blk.min(axis=1)).astype(np.float32)
        mxc[mxc == 0.0] = 1.0
        np.multiply(blk, (np.float32(127.0) / mxc)[:, None], out=tmp)
        np.add(tmp, MAGIC, out=tmp)
        np.copyto(bc[OFF_BIAS:OFF_BIAS + SZ_BIAS].view(np.int8)
                  .reshape(S, S), tmp.view(np.int32), casting='unsafe')
        # scales, tiled [p][t]: q = t*128 + p
        sc_t = (mxc / np.float32(127.0)).reshape(S // 128, 128).T
        bc[OFF_BSC:OFF_BSC + SZ_BSC].view(np.float32)[:] = sc_t.ravel()

        xbf = bc[OFF_XS:OFF_XS + SZ_XS].view(ml_dtypes.bfloat16)
        xbf.reshape(RPC, D)[:] = x2[c * RPC:(c + 1) * RPC]

        bc[OFF_WQK:OFF_WQK + SZ_WQK].view(ml_dtypes.bfloat16)[:] = \
            wqkT_t[c].ravel()
        bc[OFF_WV:OFF_WV + SZ_WV].view(ml_dtypes.bfloat16)[:] = \
            wvT_t[c].ravel()
        bc[OFF_BQK:OFF_BQK + SZ_BQK].view(np.float32)[:] = bqk[c]
        bc[OFF_BV:OFF_BV + SZ_BV].view(np.float32)[:] = bv[c]
        bc[OFF_WOS:OFF_WOS + SZ_WOS].view(ml_dtypes.bfloat16)[:] = \
            woT[c].ravel()
        bc[OFF_BO:OFF_BO + SZ_BO].view(np.float32)[:] = bo
        chunks.append(jax.device_put(bc.reshape(1, BYTES_PC),
                                     run.devices[c]))

    blob_g = jax.make_array_from_single_device_arrays(
        (N_CORES, BYTES_PC), run.sharding, chunks)

    outs = run({"blob": blob_g})
    o = outs["out"]
    shard0 = min(o.addressable_shards, key=lambda s: s.index[0].start or 0)
    return (np.asarray(shard0.data).astype(np.float32)
            .reshape(B, S, D))
